# revision 7
# baseline (speedup 1.0000x reference)
"""Trainium2 Bass kernel for nn_Discriminator (AdderNet CNN, 5 layers), v2.

Per core (batch-sharded 256/8=32):
  adder2d(x,W) = -sum_d |p_d - w_d| = -S1 + SW + 2*M2
    S1 via block(-1) patch matmuls (shared by all co per psum tile)
    SW folded into the drain bias (host constant per psum row)
    M2 = sum_d min(p_d - w_d, 0), elementwise split across three engines:
      DVE : tensor_scalar (sub, min) -> bf16, one-hot +2 bf16 matmul
      ACT : Relu(w - p)              -> fp8, one-hot -2 fp8 DoubleRow matmul
      Pool: tensor_scalar (sub, min) -> fp8, one-hot +2 fp8 DoubleRow matmul
    fp8 DoubleRow pairs two khkw-planes per matmul (2x fewer streams at
    0.5 cyc/row); L1 is fp8-sensitive and stays all-bf16.
  Layout (L2-L5): partitions = (img-group g, ci); psum rows = (g, co%Ci);
  patch tiles are full-128-partition strided copies from the padded
  activation buffers h1p..h4p. Training-mode BN via accum_out stats,
  [C,2] AllReduce, coeffs on device, Prelu applied in-place on the padded
  buffers (pads stay zero).
"""
import numpy as np
import ml_dtypes

NCORES = 8
NPC = 32
EPS = 1e-5
SLOPE = 0.2
BF = ml_dtypes.bfloat16

_cache = {}


def _install_bir_fix():
    """walrus workaround: ISA allows 1 sync-wait per instruction (2 for
    EventSemaphore); hoist excess waits onto injected EventSemaphores."""
    import orjson
    import concourse.bass_utils as bu
    import concourse.bass2jax as b2j

    if getattr(bu.compile_bir_kernel, "_waitfix", False):
        return

    def _fix(bir_json):
        bir = orjson.loads(bir_json)
        mods = bir.get("modules") or [bir]
        n = 0
        changed = False
        for mod in mods:
            for fn in mod.get("functions", []):
                for blk in fn.get("blocks", []):
                    out = []
                    for ins in blk.get("instructions", []):
                        cap = 2 if ins.get("opcode") == "EventSemaphore" else 1
                        waits = ins.get("sync_info", {}).get("on_wait", [])
                        if len(waits) > cap:
                            changed = True
                            for w in waits[:-cap]:
                                n += 1
                                out.append({
                                    "engine": ins["engine"], "ins": [], "outs": [],
                                    "name": f"I-waitfix-{n}",
                                    "opcode": "EventSemaphore",
                                    "sync_info": {"on_update": [], "on_wait": [w]},
                                    **({"debug": ins["debug"]} if "debug" in ins else {}),
                                })
                            ins["sync_info"]["on_wait"] = waits[-cap:]
                        out.append(ins)
                    blk["instructions"] = out
        return orjson.dumps(bir) if changed else bir_json

    orig = bu.compile_bir_kernel

    def wrapped(bir_json, tmpdir, neff_name="file.neff"):
        return orig(_fix(bir_json), tmpdir, neff_name)

    wrapped._waitfix = True
    bu.compile_bir_kernel = wrapped
    b2j.compile_bir_kernel = wrapped


# layer geometry (L2-L5): partitions (G img-groups x Ci), psum rows (g, c)
# with co = c + Ci*h.  KK = kernel positions, mb_n = imgs per group per batch.
LCFG = {
    2: dict(Ci=16, Co=32, K=4, Ho=32, G=8, KK=16, mb_n=1, n_mb=4,
            pad_hw=34, nsub=2),
    3: dict(Ci=32, Co=64, K=3, Ho=16, G=4, KK=9, mb_n=8, n_mb=1,
            pad_hw=18, nsub=4),
    4: dict(Ci=64, Co=128, K=4, Ho=8, G=2, KK=16, mb_n=16, n_mb=1,
            pad_hw=10, nsub=2),
    5: dict(Ci=128, Co=1, K=4, Ho=4, G=1, KK=16, mb_n=32, n_mb=1,
            pad_hw=None, nsub=1),
}
CNT = {1: 256 * 64 * 64, 2: 256 * 32 * 32, 3: 256 * 16 * 16,
       4: 256 * 8 * 8, 5: 256 * 4 * 4}
NCH = {1: 16, 2: 32, 3: 64, 4: 128, 5: 1}
NTILES = {1: 32, 2: 16, 3: 8, 4: 4, 5: 1}

# engine striping (D=DVE bf16, A=ACT fp8, P=Pool fp8, E=DVE fp8)
STRIPE_W = {1: (6, 2, 2, 0), 2: (11, 5, 3, 1), 3: (10, 5, 3, 2),
            4: (8, 5, 3, 4), 5: (1, 0, 0, 0)}


def _mkpat(weights):
    total = sum(weights)
    letters = "DAPE"
    acc = [0] * 4
    seq = []
    for _ in range(total):
        for i in range(4):
            acc[i] += weights[i]
        j = max(range(4), key=lambda i: acc[i])
        acc[j] -= total
        seq.append(letters[j])
    return "".join(seq)


PATS = {l: _mkpat(w) for l, w in STRIPE_W.items()}


def _engine(l, idx):
    p = PATS[l]
    return p[idx % len(p)]


def _build(taps=()):
    import contextlib
    import concourse.bass as bass
    import concourse.mybir as mybir
    from concourse.tile import TileContext

    F32 = mybir.dt.float32
    BF16 = mybir.dt.bfloat16
    FP8 = mybir.dt.float8e4
    A = mybir.AluOpType
    AF = mybir.ActivationFunctionType
    AX = mybir.AxisListType
    DR = mybir.MatmulPerfMode.DoubleRow
    F8 = ml_dtypes.float8_e4m3fn

    nc = bass.Bass(num_devices=NCORES)

    # ---------------- dram inputs ----------------
    p1_d = nc.dram_tensor("p1", [128, 16384], BF16, kind="ExternalInput")
    w1rep_d = nc.dram_tensor("w1rep", [128, 16], F32, kind="ExternalInput")
    swb_d = {1: nc.dram_tensor("swb1", [128, 1], F32, kind="ExternalInput")}
    wsc_d = {}
    for l in (2, 3, 4, 5):
        cfg = LCFG[l]
        wsc_d[l] = nc.dram_tensor(f"w{l}sc", [128, cfg["KK"] * cfg["Co"]], F32,
                                  kind="ExternalInput")
        shp = [1, 1] if l == 5 else [128, 2]
        swb_d[l] = nc.dram_tensor(f"swb{l}", shp, F32, kind="ExternalInput")
    gb_d = {l: nc.dram_tensor(f"gb{l}", [NCH[l], 2], F32, kind="ExternalInput")
            for l in (1, 2, 3, 4, 5)}
    out_d = nc.dram_tensor("out", [1, 512], F32, kind="ExternalOutput")
    tap_d = {}
    for t in taps:
        shp = {"h1": [128, 4 * 66 * 66], "h2": [128, 8 * 34 * 34],
               "h3": [128, 16 * 18 * 18], "h4": [128, 32 * 10 * 10],
               "raw5": [1, 512]}[t]
        tap_d[t] = nc.dram_tensor("tap_" + t, shp,
                                  F32 if t == "raw5" else BF16,
                                  kind="ExternalOutput")

    cc_in = {l: nc.dram_tensor(f"cci{l}", [NCH[l], 2], F32, kind="Internal")
             for l in (1, 5)}
    cc_out = {l: nc.dram_tensor(f"cco{l}", [NCH[l], 2], F32, kind="Internal",
                                addr_space="Shared")
              for l in (1, 5)}
    for l in (2, 3, 4):
        CB = LCFG[l]["Ci"]
        for h in range(NCH[l] // CB):
            cc_in[(l, h)] = nc.dram_tensor(f"cci{l}_{h}", [CB, 2], F32,
                                           kind="Internal")
            cc_out[(l, h)] = nc.dram_tensor(f"cco{l}_{h}", [CB, 2], F32,
                                            kind="Internal", addr_space="Shared")

    # ---------------- inline constants ----------------
    # one-hot families; CB = Ci (row-block per group)
    def onehot_bf(CB, G, val):
        m = np.zeros((128, 256), BF)
        for k in range(128):
            m[k, 128 + CB * (k // CB)] = BF(val)
        return m

    def onehot_f8(CB, G, val):
        m = np.zeros((128, 640), F8)
        for k in range(128):
            m[k, 128 + CB * (k // CB)] = F8(val)
            m[k, 384 + CB * (k // CB)] = F8(val)
        return m

    def blockones(CB):
        m = np.zeros((128, 128), BF)
        for k in range(128):
            m[k, CB * (k // CB):CB * (k // CB) + CB] = BF(-1.0)
        return m

    # L1: partitions (band b, d16): same CB=16 structure, one-hots +-2
    oh1p = onehot_bf(16, 8, 2.0)
    oh1n = onehot_bf(16, 8, -2.0)
    b16 = blockones(16)          # shared by L1 and L2
    b32 = blockones(32)
    b64 = blockones(64)
    b1 = np.full((128, 1), -1.0, BF)   # L5 S1
    ohp = {2: onehot_bf(16, 8, 2.0), 3: onehot_bf(32, 4, 2.0),
           4: onehot_bf(64, 2, 2.0)}
    ohn = {2: onehot_bf(16, 8, -2.0), 3: onehot_bf(32, 4, -2.0),
           4: onehot_bf(64, 2, -2.0)}
    odp = {2: onehot_f8(16, 8, 2.0), 3: onehot_f8(32, 4, 2.0),
           4: onehot_f8(64, 2, 2.0)}
    odn = {2: onehot_f8(16, 8, -2.0), 3: onehot_f8(32, 4, -2.0),
           4: onehot_f8(64, 2, -2.0)}
    oh5p = np.full((128, 1), 2.0, BF)
    oh5n = np.full((128, 1), -2.0, BF)
    od5p = np.zeros((128, 32), F8); od5p[:, 0] = F8(2.0); od5p[:, 16] = F8(2.0)
    od5n = np.zeros((128, 32), F8); od5n[:, 0] = F8(-2.0); od5n[:, 16] = F8(-2.0)

    # stats folds sf[l][h]: [128, C] one-hot (16g+c -> c+CB*h etc.)
    sfm = {}
    for l in (1, 2, 3, 4):
        CB = NCH[1] if l == 1 else LCFG[l]["Ci"]
        C = NCH[l]
        halves = 1 if l == 1 else C // CB
        ms = []
        for h in range(halves):
            m = np.zeros((128, C), np.float32)
            for k in range(128):
                m[k, (k % CB) + CB * h] = 1.0
            ms.append(m)
        sfm[l] = ms
    # coeff replication rep[l]: [C, 128]: k -> k % C (l=4 identity, skip)
    repm = {}
    for l, C in ((1, 16), (2, 32), (3, 64)):
        r = np.zeros((C, 128), np.float32)
        for k in range(128):
            r[k % C, k] = 1.0
        repm[l] = r

    inl = lambda nm, a: nc.inline_tensor(np.ascontiguousarray(a), name=nm)
    oh1p_t, oh1n_t = inl("c_oh1p", oh1p), inl("c_oh1n", oh1n)
    b16_t, b32_t, b64_t, b1_t = inl("c_b16", b16), inl("c_b32", b32), \
        inl("c_b64", b64), inl("c_b1", b1)
    ohp_t = {l: inl(f"c_ohp{l}", ohp[l]) for l in ohp}
    ohn_t = {l: inl(f"c_ohn{l}", ohn[l]) for l in ohn}
    odp_t = {l: inl(f"c_odp{l}", odp[l]) for l in odp}
    odn_t = {l: inl(f"c_odn{l}", odn[l]) for l in odn}
    oh5p_t, oh5n_t = inl("c_oh5p", oh5p), inl("c_oh5n", oh5n)
    od5p_t, od5n_t = inl("c_od5p", od5p), inl("c_od5n", od5n)
    sf_t = {l: [inl(f"c_sf{l}_{h}", m) for h, m in enumerate(sfm[l])]
            for l in sfm}
    rep_t = {l: inl(f"c_rep{l}", repm[l]) for l in repm}

    with TileContext(nc) as tc:
        with contextlib.ExitStack() as ctx:
            cp = ctx.enter_context(tc.tile_pool(name="consts", bufs=1))
            hp = ctx.enter_context(tc.tile_pool(name="hpads", bufs=1))
            sp = ctx.enter_context(tc.tile_pool(name="small", bufs=1))
            rawp = ctx.enter_context(tc.tile_pool(name="rawc", bufs=4))
            sqp = ctx.enter_context(tc.tile_pool(name="sq", bufs=2))
            ps = ctx.enter_context(tc.tile_pool(name="psum", bufs=6, space="PSUM"))
            ps2 = ctx.enter_context(tc.tile_pool(name="psum2", bufs=2, space="PSUM"))

            def load_const(tag, dram, shape, dtype):
                t = cp.tile(shape, dtype, name=tag, tag=tag)
                nc.sync.dma_start(t[:], dram[:])
                return t

            oh1p_s = load_const("oh1p", oh1p_t, [128, 256], BF16)
            oh1n_s = load_const("oh1n", oh1n_t, [128, 256], BF16)
            b16_s = load_const("b16", b16_t, [128, 128], BF16)
            b32_s = load_const("b32", b32_t, [128, 128], BF16)
            b64_s = load_const("b64", b64_t, [128, 128], BF16)
            b1_s = load_const("b1", b1_t, [128, 1], BF16)
            bl_s = {1: b16_s, 2: b16_s, 3: b32_s, 4: b64_s, 5: b1_s}
            ohp_s = {l: load_const(f"ohp{l}", ohp_t[l], [128, 256], BF16) for l in ohp}
            ohn_s = {l: load_const(f"ohn{l}", ohn_t[l], [128, 256], BF16) for l in ohn}
            odp_s = {l: load_const(f"odp{l}", odp_t[l], [128, 640], FP8) for l in odp}
            odn_s = {l: load_const(f"odn{l}", odn_t[l], [128, 640], FP8) for l in odn}
            oh5p_s = load_const("oh5p", oh5p_t, [128, 1], BF16)
            oh5n_s = load_const("oh5n", oh5n_t, [128, 1], BF16)
            od5p_s = load_const("od5p", od5p_t, [128, 32], FP8)
            od5n_s = load_const("od5n", od5n_t, [128, 32], FP8)
            sf_s = {l: [load_const(f"sf{l}_{h}", t, [128, NCH[l]], F32)
                        for h, t in enumerate(sf_t[l])] for l in sf_t}
            rep_s = {l: load_const(f"rep{l}", rep_t[l], [NCH[l], 128], F32)
                     for l in rep_t}
            w1rep_s = load_const("w1rep", w1rep_d, [128, 16], F32)
            wscp = ctx.enter_context(tc.tile_pool(name="wscp", bufs=1))

            def load_wsc(l):
                t = wscp.tile([128, 2048], F32, name=f"wsc{l}", tag="wsc")
                nc.sync.dma_start(t[:, 0:LCFG[l]["KK"] * LCFG[l]["Co"]],
                                  wsc_d[l][:])
                return t
            swb_s = {l: load_const(f"swb{l}", swb_d[l],
                                   [1, 1] if l == 5 else
                                   ([128, 1] if l == 1 else [128, 2]), F32)
                     for l in (1, 2, 3, 4, 5)}
            gb_s = {l: load_const(f"gb{l}", gb_d[l], [NCH[l], 2], F32)
                    for l in (1, 2, 3, 4, 5)}

            hpA = ctx.enter_context(tc.tile_pool(name="hpA", bufs=1))
            hpB = ctx.enter_context(tc.tile_pool(name="hpB", bufs=1))

            def new_h(slot, dims):
                pool, width = (hpA, 17424) if slot == "A" else (hpB, 9248)
                t = pool.tile([128, width], BF16, name="h" + slot, tag="h" + slot)
                nc.gpsimd.memset(t[:], 0.0)
                a, b, c = dims
                return t, t[:, 0:a * b * c].rearrange("p (a b c) -> p a b c", a=a, b=b, c=c)

            h1f, h1p = new_h("A", (4, 66, 66))

            st_s = {l: sp.tile([128 if l < 5 else 1, NTILES[l]], F32,
                               name=f"sts{l}", tag=f"sts{l}") for l in NTILES}
            st_q = {l: sp.tile([128 if l < 5 else 1, NTILES[l]], F32,
                               name=f"stq{l}", tag=f"stq{l}") for l in NTILES}

            # ---------------- BN coeffs (a, c) ----------------
            def bn_stats_half(l, h):
                CB = LCFG[l]["Ci"]
                nt = NTILES[l]
                halves = NCH[l] // CB
                f = sp.tile([128, 2], F32, name=f"stf{l}_{h}", tag=f"stf{l}_{h}")
                sl = slice(h * (nt // halves), (h + 1) * (nt // halves))
                nc.vector.tensor_reduce(f[:, 0:1], st_s[l][:, sl], AX.X, A.add)
                nc.vector.tensor_reduce(f[:, 1:2], st_q[l][:, sl], AX.X, A.add)
                psf = ps2.tile([CB, 2], F32, name="paux", tag="paux")
                nc.tensor.matmul(psf[:], sf_s[l][0][:, 0:CB], f[:],
                                 start=True, stop=True)
                stc = sp.tile([CB, 2], F32, name=f"stc{l}_{h}", tag=f"stc{l}_{h}")
                nc.scalar.copy(stc[:], psf[:])
                nc.sync.dma_start(cc_in[(l, h)][:], stc[:])
                nc.gpsimd.collective_compute(
                    "AllReduce", A.add, replica_groups=[list(range(NCORES))],
                    ins=[cc_in[(l, h)][:]], outs=[cc_out[(l, h)][:]])

            def bn_finish(l):
                C = NCH[l]
                CB = LCFG[l]["Ci"]
                R = 128
                rr = sp.tile([C, 4], F32, name=f"rr{l}", tag=f"rr{l}")
                for h in range(C // CB):
                    nc.sync.dma_start(rr[CB * h:CB * (h + 1), 0:2],
                                      cc_out[(l, h)][:])
                nc.vector.tensor_copy(rr[:, 2:4], gb_s[l][:])
                nb = sp.tile([R, 4], F32, name=f"nb{l}", tag=f"nb{l}")
                if l in rep_t:
                    prr = ps2.tile([128, 4], F32, name="paux", tag="paux")
                    nc.tensor.matmul(prr[:], rep_s[l][:], rr[:], start=True,
                                     stop=True)
                    nc.scalar.copy(nb[:], prr[:])
                else:
                    nc.vector.tensor_copy(nb[:], rr[:])
                return _coeff_math(l, R, nb)

            def _coeff_math(l, R, nb):
                ic = 1.0 / CNT[l]
                mS = sp.tile([R, 1], F32, name=f"mS{l}", tag=f"mS{l}")
                v = sp.tile([R, 1], F32, name=f"v{l}", tag=f"v{l}")
                nc.vector.tensor_scalar_mul(mS[:], nb[:, 0:1], ic)
                nc.vector.tensor_tensor(v[:], mS[:], mS[:], A.mult)
                mQ = sp.tile([R, 1], F32, name=f"mQ{l}", tag=f"mQ{l}")
                nc.vector.tensor_scalar_mul(mQ[:], nb[:, 1:2], ic)
                nc.vector.tensor_tensor(v[:], mQ[:], v[:], A.subtract)
                nc.vector.tensor_scalar_add(v[:], v[:], EPS)
                y0 = sp.tile([R, 1], F32, name=f"y0{l}", tag=f"y0{l}")
                nc.scalar.activation(y0[:], v[:], AF.Sqrt)
                r0 = sp.tile([R, 1], F32, name=f"r0{l}", tag=f"r0{l}")
                nc.vector.reciprocal(r0[:], y0[:])
                t0 = sp.tile([R, 1], F32, name=f"t0{l}", tag=f"t0{l}")
                nc.vector.tensor_tensor(t0[:], v[:], r0[:], A.mult)
                nc.vector.tensor_tensor(t0[:], y0[:], t0[:], A.add)
                nc.vector.tensor_scalar_mul(t0[:], t0[:], 0.5)
                rsq = sp.tile([R, 1], F32, name=f"rsq{l}", tag=f"rsq{l}")
                nc.vector.reciprocal(rsq[:], t0[:])
                a = sp.tile([R, 1], F32, name=f"a{l}", tag=f"a{l}")
                nc.vector.tensor_tensor(a[:], nb[:, 2:3], rsq[:], A.mult)
                c = sp.tile([R, 1], F32, name=f"c{l}", tag=f"c{l}")
                nc.vector.tensor_tensor(c[:], mS[:], a[:], A.mult)
                nc.vector.tensor_tensor(c[:], nb[:, 3:4], c[:], A.subtract)
                return a, c

            def bn_coeffs(l):
                C = NCH[l]
                R = 128 if l < 5 else 1
                halves = 1 if l in (1, 5) else C // LCFG[l]["Ci"]
                nt = NTILES[l]
                stf = []
                for h in range(halves):
                    f = sp.tile([R, 2], F32, name=f"stf{l}_{h}", tag=f"stf{l}_{h}")
                    sl = slice(h * (nt // halves), (h + 1) * (nt // halves))
                    nc.vector.tensor_reduce(f[:, 0:1], st_s[l][:, sl], AX.X, A.add)
                    nc.vector.tensor_reduce(f[:, 1:2], st_q[l][:, sl], AX.X, A.add)
                    stf.append(f)
                stc = sp.tile([C, 2], F32, name=f"stc{l}", tag=f"stc{l}")
                if l < 5:
                    psf = ps2.tile([C, 2], F32, name="paux", tag="paux")
                    for h in range(halves):
                        nc.tensor.matmul(psf[:], sf_s[l][h][:], stf[h][:],
                                         start=(h == 0), stop=(h == halves - 1))
                    nc.scalar.copy(stc[:], psf[:])
                else:
                    nc.vector.tensor_copy(stc[:], stf[0][:])
                nc.sync.dma_start(cc_in[l][:], stc[:])
                nc.gpsimd.collective_compute(
                    "AllReduce", A.add, replica_groups=[list(range(NCORES))],
                    ins=[cc_in[l][:]], outs=[cc_out[l][:]])
                nb = sp.tile([R, 4], F32, name=f"nb{l}", tag=f"nb{l}")
                if l in rep_t:
                    rr = sp.tile([C, 4], F32, name=f"rr{l}", tag=f"rr{l}")
                    nc.sync.dma_start(rr[:, 0:2], cc_out[l][:])
                    nc.vector.tensor_copy(rr[:, 2:4], gb_s[l][:])
                    prr = ps2.tile([128, 4], F32, name="paux", tag="paux")
                    nc.tensor.matmul(prr[:], rep_s[l][:], rr[:], start=True, stop=True)
                    nc.scalar.copy(nb[:], prr[:])
                else:
                    nc.sync.dma_start(nb[:, 0:2], cc_out[l][:])
                    nc.vector.tensor_copy(nb[:, 2:4], gb_s[l][:])
                ic = 1.0 / CNT[l]
                mS = sp.tile([R, 1], F32, name=f"mS{l}", tag=f"mS{l}")
                v = sp.tile([R, 1], F32, name=f"v{l}", tag=f"v{l}")
                nc.vector.tensor_scalar_mul(mS[:], nb[:, 0:1], ic)
                nc.vector.tensor_tensor(v[:], mS[:], mS[:], A.mult)
                mQ = sp.tile([R, 1], F32, name=f"mQ{l}", tag=f"mQ{l}")
                nc.vector.tensor_scalar_mul(mQ[:], nb[:, 1:2], ic)
                nc.vector.tensor_tensor(v[:], mQ[:], v[:], A.subtract)
                nc.vector.tensor_scalar_add(v[:], v[:], EPS)
                y0 = sp.tile([R, 1], F32, name=f"y0{l}", tag=f"y0{l}")
                nc.scalar.activation(y0[:], v[:], AF.Sqrt)
                r0 = sp.tile([R, 1], F32, name=f"r0{l}", tag=f"r0{l}")
                nc.vector.reciprocal(r0[:], y0[:])
                t0 = sp.tile([R, 1], F32, name=f"t0{l}", tag=f"t0{l}")
                nc.vector.tensor_tensor(t0[:], v[:], r0[:], A.mult)
                nc.vector.tensor_tensor(t0[:], y0[:], t0[:], A.add)
                nc.vector.tensor_scalar_mul(t0[:], t0[:], 0.5)
                rsq = sp.tile([R, 1], F32, name=f"rsq{l}", tag=f"rsq{l}")
                nc.vector.reciprocal(rsq[:], t0[:])
                a = sp.tile([R, 1], F32, name=f"a{l}", tag=f"a{l}")
                nc.vector.tensor_tensor(a[:], nb[:, 2:3], rsq[:], A.mult)
                c = sp.tile([R, 1], F32, name=f"c{l}", tag=f"c{l}")
                nc.vector.tensor_tensor(c[:], mS[:], a[:], A.mult)
                nc.vector.tensor_tensor(c[:], nb[:, 3:4], c[:], A.subtract)
                return a, c

            def drain(l, pt, tile_idx, bias_col, scatter_fn):
                """psum -> rawc (bias+stats), sq stats, scatter DMAs."""
                R = 128 if l < 5 else 1
                rawc = rawp.tile([R, 512], BF16 if l < 5 else F32,
                                 name=f"raw{l}", tag=f"rawc{l}")
                nc.scalar.activation(rawc[:], pt[0:R, :], AF.Identity,
                                     bias=bias_col,
                                     accum_out=st_s[l][:, tile_idx:tile_idx + 1])
                sq = sqp.tile([R, 512], BF16 if l < 5 else F32,
                              name=f"sq{l}", tag=f"sqc{l}")
                nc.vector.scalar_tensor_tensor(
                    sq[:], rawc[:], 1.0, rawc[:], A.mult, A.mult,
                    accum_out=st_q[l][:, tile_idx:tile_idx + 1])
                scatter_fn(rawc)
                return rawc

            # ---------------- Layer 1 ----------------
            # p1 [128=(band b, d), 16384=(img a, 512)] bf16; psum rows 16b+co
            with tc.tile_pool(name="l1p", bufs=3) as pp1, \
                 tc.tile_pool(name="l1d", bufs=6) as dp1:
                uidx = 0
                for ch in range(8):
                    p1c = pp1.tile([128, 2048], BF16, name="p1c", tag="p1c")
                    nc.sync.dma_start(p1c[:], p1_d[:, ch * 2048:(ch + 1) * 2048])
                    pts = [ps.tile([128, 512], F32, name="pmain", tag="pmain")
                           for _ in range(4)]
                    for tt in range(4):
                        nc.tensor.matmul(pts[tt][:], b16_s[:],
                                         p1c[:, tt * 512:(tt + 1) * 512],
                                         start=True, stop=False)
                    for co in range(16):
                        eng = _engine(1, uidx); uidx += 1
                        d1 = dp1.tile([128, 2048], BF16, name="d1", tag="d1")
                        if eng == "A":
                            nc.scalar.activation(d1[:], p1c[:], AF.Relu,
                                                 bias=w1rep_s[:, co:co + 1],
                                                 scale=-1.0)
                            oh = oh1n_s
                        else:
                            e = nc.vector if eng == "D" else nc.gpsimd
                            e.tensor_scalar(d1[:], p1c[:], w1rep_s[:, co:co + 1],
                                            0.0, A.subtract, A.min)
                            oh = oh1p_s
                        for tt in range(4):
                            nc.tensor.matmul(pts[tt][:],
                                             oh[:, 128 - co:256 - co],
                                             d1[:, tt * 512:(tt + 1) * 512],
                                             start=False, stop=(co == 15))
                    for tt in range(4):
                        a_img = ch * 4 + tt

                        def sc1(rawc, a_img=a_img):
                            for b in range(8):
                                nc.sync.dma_start(
                                    h1p[16 * (a_img // 4):16 * (a_img // 4) + 16,
                                        a_img % 4, 1 + 8 * b:9 + 8 * b, 1:65],
                                    rawc[16 * b:16 * b + 16, :])
                        drain(1, pts[tt], ch * 4 + tt, swb_s[1][:, 0:1], sc1)

                a1, c1 = bn_coeffs(1)
                if "h1" in tap_d:
                    nc.sync.dma_start(tap_d["h1"][:], h1f[:, 0:17424])

            # ---------------- Layers 2-4 ----------------
            def patch_fix(t, K, Ho, k):
                kh, kw = k // K, k % K
                if kh == 0:
                    nc.vector.tensor_scalar_mul(t[:, :, 0:1, :], t[:, :, 0:1, :], 0.0)
                if kh == 3:
                    nc.vector.tensor_scalar_mul(t[:, :, Ho - 1:Ho, :],
                                                t[:, :, Ho - 1:Ho, :], 0.0)
                if kw == 0:
                    nc.vector.tensor_scalar_mul(t[:, :, :, 0:1], t[:, :, :, 0:1], 0.0)
                if kw == 3:
                    nc.vector.tensor_scalar_mul(t[:, :, :, Ho - 1:Ho],
                                                t[:, :, :, Ho - 1:Ho], 0.0)

            def run_layer(l, src, dst, scatter, pre):
                cfg = LCFG[l]
                Ci, Co, K, Ho = cfg["Ci"], cfg["Co"], cfg["K"], cfg["Ho"]
                G, KK, mb_n, nsub = cfg["G"], cfg["KK"], cfg["mb_n"], cfg["nsub"]
                halves = Co // Ci
                F = mb_n * Ho * Ho
                npair = KK // 2
                odd = KK % 2
                wsc_l = load_wsc(l)
                nd, na, ng, ne = {2: (8, 6, 4, 2), 3: (4, 4, 3, 2)}.get(
                    l, (6, 6, 4, 3))
                ns1 = 4
                with tc.tile_pool(name=f"l{l}p", bufs=1) as pp, \
                     tc.tile_pool(name=f"l{l}d", bufs=nd) as dp, \
                     tc.tile_pool(name=f"l{l}a", bufs=na) as ap, \
                     tc.tile_pool(name=f"l{l}g", bufs=ng) as gp, \
                     tc.tile_pool(name=f"l{l}s", bufs=ns1) as s1p, \
                     tc.tile_pool(name=f"l{l}e", bufs=ne) as ep:
                    uidx = 0
                    for mb in range(cfg["n_mb"]):
                        ptk = []
                        for k in range(KK):
                            kh, kw = k // K, k % K
                            t = pp.tile([128, mb_n, Ho, Ho], BF16,
                                        name=f"p{l}_{k}", tag=f"p{l}_{k}")
                            nc.scalar.activation(
                                t[:],
                                src[:, mb * mb_n:(mb + 1) * mb_n,
                                    kh:kh + 2 * Ho - 1:2, kw:kw + 2 * Ho - 1:2],
                                AF.Prelu, bias=pre[1][:, 0:1],
                                scale=pre[0][:, 0:1], alpha=SLOPE)
                            patch_fix(t, K, Ho, k)
                            ptk.append(t)
                        ptf = [t[:].rearrange("p a b c -> p (a b c)") for t in ptk]
                        s1bs = []
                        for q in range(nsub):
                            ps1 = ps.tile([128, 512], F32, name="pmain",
                                          tag="pmain")
                            for k in range(KK):
                                nc.tensor.matmul(
                                    ps1[:], bl_s[l][:],
                                    ptf[k][:, q * 512:(q + 1) * 512],
                                    start=(k == 0), stop=(k == KK - 1))
                            s1b = s1p.tile([128, 512], F32, name=f"s1b{l}",
                                           tag=f"s1b{l}")
                            nc.scalar.copy(s1b[:], ps1[:])
                            s1bs.append(s1b)
                        for h in range(halves):
                            pts = [ps.tile([128, 512], F32, name="pmain",
                                           tag="pmain") for _ in range(nsub)]
                            first = [True] * nsub
                            for c in range(Ci):
                                co = c + Ci * h
                                for t in range(npair + odd):
                                    single = (t == npair)
                                    eng = "D" if single else _engine(l, uidx)
                                    uidx += 1
                                    k0, k1 = 2 * t, 2 * t + 1
                                    last = (c == Ci - 1) and (t == npair + odd - 1)
                                    if single or eng == "D":
                                        ks = [k0] if single else [k0, k1]
                                        for ki, k in enumerate(ks):
                                            dl = dp.tile([128, F], BF16,
                                                         name=f"dl{l}", tag=f"dl{l}")
                                            nc.vector.tensor_scalar(
                                                dl[:], ptf[k][:],
                                                wsc_l[:, k * Co + co:k * Co + co + 1],
                                                0.0, A.subtract, A.min)
                                            lst = last and (ki == len(ks) - 1)
                                            for q in range(nsub):
                                                nc.tensor.matmul(
                                                    pts[q][:],
                                                    ohp_s[l][:, 128 - c:256 - c],
                                                    dl[:, q * 512:(q + 1) * 512],
                                                    start=first[q],
                                                    stop=(lst and q == nsub - 1))
                                                first[q] = False
                                    else:
                                        pool_ = {"A": ap, "P": gp, "E": ep}[eng]
                                        dl = pool_.tile([128, 2, F], FP8,
                                                        name=f"dh{l}", tag=f"dh{l}")
                                        if eng == "A":
                                            for j, k in ((0, k0), (1, k1)):
                                                nc.scalar.activation(
                                                    dl[:, j, :], ptf[k][:], AF.Relu,
                                                    bias=wsc_l[:, k * Co + co:k * Co + co + 1],
                                                    scale=-1.0)
                                            ohw = odn_s[l]
                                        else:
                                            e = nc.gpsimd if eng == "P" else nc.vector
                                            for j, k in ((0, k0), (1, k1)):
                                                e.tensor_scalar(
                                                    dl[:, j, :], ptf[k][:],
                                                    wsc_l[:, k * Co + co:k * Co + co + 1],
                                                    0.0, A.subtract, A.min)
                                            ohw = odp_s[l]
                                        lhsT = ohw[:, 128 - c:128 - c + 512].rearrange(
                                            "p (j m) -> p j m", j=2)[:, :, 0:128]
                                        for q in range(nsub):
                                            nc.tensor.matmul(
                                                pts[q][:], lhsT,
                                                dl[:, :, q * 512:(q + 1) * 512],
                                                start=first[q],
                                                stop=(last and q == nsub - 1),
                                                perf_mode=DR)
                                            first[q] = False
                            for q in range(nsub):
                                tile_idx = h * (NTILES[l] // halves) + \
                                    mb * nsub + q
                                rawc = rawp.tile([128, 512], BF16,
                                                 name=f"raw{l}", tag=f"rawc{l}")
                                nc.vector.scalar_tensor_tensor(
                                    rawc[:], pts[q][:], swb_s[l][:, h:h + 1],
                                    s1bs[q][:], A.add, A.add,
                                    accum_out=st_s[l][:, tile_idx:tile_idx + 1])
                                sq = sqp.tile([128, 512], BF16,
                                              name=f"sq{l}", tag=f"sqc{l}")
                                nc.vector.scalar_tensor_tensor(
                                    sq[:], rawc[:], 1.0, rawc[:], A.mult, A.mult,
                                    accum_out=st_q[l][:, tile_idx:tile_idx + 1])
                                scatter(rawc, mb, h, q)
                            if mb == cfg["n_mb"] - 1:
                                bn_stats_half(l, h)

            # L2
            def sc2(rawc, mb, h, q):
                for g in range(8):
                    mprime = 4 * (g % 2) + mb
                    nc.sync.dma_start(
                        h2p[32 * (g // 2) + 16 * h:32 * (g // 2) + 16 * h + 16,
                            mprime, 1 + 16 * q:17 + 16 * q, 1:33],
                        rawc[16 * g:16 * g + 16, :])

            h2f, h2p = new_h("B", (8, 34, 34))
            run_layer(2, h1p, h2p, sc2, (a1, c1))
            a2, c2 = bn_finish(2)
            if "h2" in tap_d:
                nc.sync.dma_start(tap_d["h2"][:], h2f[:, 0:9248])

            # L3
            def sc3(rawc, mb, h, q):
                for g in range(4):
                    for mloc in range(2):
                        mprime = 2 * q + mloc
                        mpp = 8 * (g % 2) + mprime
                        nc.sync.dma_start(
                            h3p[64 * (g // 2) + 32 * h:64 * (g // 2) + 32 * h + 32,
                                mpp, 1:17, 1:17],
                            rawc[32 * g:32 * g + 32, mloc * 256:(mloc + 1) * 256])

            h3f, h3p = new_h("A", (16, 18, 18))
            run_layer(3, h2p, h3p, sc3, (a2, c2))
            a3, c3 = bn_finish(3)
            if "h3" in tap_d:
                nc.sync.dma_start(tap_d["h3"][:], h3f[:, 0:5184])

            # L4
            def sc4(rawc, mb, h, q):
                for g in range(2):
                    for mloc in range(8):
                        img = 16 * g + 8 * q + mloc
                        nc.sync.dma_start(
                            h4p[64 * h:64 * h + 64, img, 1:9, 1:9],
                            rawc[64 * g:64 * g + 64, mloc * 64:(mloc + 1) * 64])

            h4f, h4p = new_h("B", (32, 10, 10))
            run_layer(4, h3p, h4p, sc4, (a3, c3))
            a4, c4 = bn_finish(4)
            if "h4" in tap_d:
                nc.sync.dma_start(tap_d["h4"][:], h4f[:, 0:3200])

            # ---------------- Layer 5 ----------------
            with tc.tile_pool(name="l5p", bufs=1) as pp5, \
                 tc.tile_pool(name="l5d", bufs=4) as dp5:
                wsc_5 = load_wsc(5)
                ptk = []
                for k in range(16):
                    kh, kw = k // 4, k % 4
                    t = pp5.tile([128, 32, 4, 4], BF16, name=f"p5_{k}",
                                 tag=f"p5_{k}")
                    nc.scalar.activation(t[:], h4p[:, :, kh:kh + 7:2, kw:kw + 7:2],
                                         AF.Prelu, bias=c4[:, 0:1],
                                         scale=a4[:, 0:1], alpha=SLOPE)
                    patch_fix(t, 4, 4, k)
                    ptk.append(t)
                ptf = [t[:].rearrange("p a b c -> p (a b c)") for t in ptk]
                pt = ps.tile([128, 512], F32, name="pmain", tag="pmain")
                for k in range(16):
                    nc.tensor.matmul(pt[0:1, :], b1_s[:], ptf[k][:],
                                     start=(k == 0), stop=False)
                for k in range(16):
                    dl = dp5.tile([128, 512], BF16, name="dl5", tag="dl5")
                    nc.vector.tensor_scalar(dl[:], ptf[k][:],
                                            wsc_5[:, k:k + 1],
                                            0.0, A.subtract, A.min)
                    nc.tensor.matmul(pt[0:1, :], oh5p_s[:], dl[:],
                                     start=False, stop=(k == 15))
                raw5 = sp.tile([1, 512], F32, name="raw5", tag="raw5")
                nc.scalar.activation(raw5[:], pt[0:1, :], AF.Identity,
                                     bias=swb_s[5][:, 0:1],
                                     accum_out=st_s[5][:, 0:1])
                sq5 = sp.tile([1, 512], F32, name="sq5", tag="sq5")
                nc.vector.scalar_tensor_tensor(
                    sq5[:], raw5[:], 1.0, raw5[:], A.mult, A.mult,
                    accum_out=st_q[5][:, 0:1])
                a5, c5 = bn_coeffs(5)
                out5 = sp.tile([1, 512], F32, name="out5", tag="out5")
                nc.scalar.activation(out5[:], raw5[:], AF.Sigmoid,
                                     bias=c5[:, 0:1], scale=a5[:, 0:1])
                if "raw5" in tap_d:
                    nc.sync.dma_start(tap_d["raw5"][:], raw5[:])
                nc.sync.dma_start(out_d[:], out5[:])

    return nc


def _host_prep(inputs):
    x = np.asarray(inputs["x"], np.float32)
    W = {l: np.asarray(inputs[f"W{l}"], np.float32) for l in (1, 2, 3, 4, 5)}
    g = {l: np.asarray(inputs[f"g{l}"], np.float32) for l in (1, 2, 3, 4, 5)}
    b = {l: np.asarray(inputs[f"b{l}"], np.float32) for l in (1, 2, 3, 4, 5)}

    W1f = W[1].reshape(16, 16)
    shared = {
        "w1rep": np.ascontiguousarray(np.tile(W1f.T, (8, 1)), np.float32),
        "swb1": np.ascontiguousarray(np.tile(W1f.sum(1), 8)[:, None], np.float32),
    }
    for l in (2, 3, 4, 5):
        cfg = LCFG[l]
        Ci, Co, K, KK, G = cfg["Ci"], cfg["Co"], cfg["K"], cfg["KK"], cfg["G"]
        # wsc [(g,ci), k*Co+co] = W[co, ci, kh, kw]
        Wk = W[l].transpose(2, 3, 1, 0).reshape(KK, Ci, Co)  # [k, ci, co]
        wsc = np.tile(Wk.reshape(KK * Ci, Co).reshape(KK, Ci, Co)
                      .transpose(1, 0, 2).reshape(Ci, KK * Co), (G, 1))
        shared[f"w{l}sc"] = np.ascontiguousarray(wsc, np.float32)
        swl = W[l].reshape(Co, -1).sum(1)  # [Co]
        if l == 5:
            shared["swb5"] = np.ascontiguousarray(swl[:, None], np.float32)
        else:
            halves = Co // Ci
            sb = np.zeros((128, halves), np.float32)
            for k in range(128):
                for h in range(halves):
                    sb[k, h] = swl[(k % Ci) + Ci * h]
            shared[f"swb{l}"] = sb
    for l in (1, 2, 3, 4, 5):
        shared[f"gb{l}"] = np.ascontiguousarray(
            np.stack([g[l].ravel(), b[l].ravel()], 1), np.float32)

    in_maps = []
    for c in range(NCORES):
        xs = x[c * NPC:(c + 1) * NPC, 0]
        xp = np.pad(xs, ((0, 0), (1, 1), (1, 1)))
        s = xp.strides
        win = np.lib.stride_tricks.as_strided(
            xp, (NPC, 64, 64, 4, 4), (s[0], 2 * s[1], 2 * s[2], s[1], s[2]))
        P1 = win.transpose(3, 4, 0, 1, 2).reshape(16, NPC * 4096)
        p1 = np.ascontiguousarray(
            P1.reshape(16, 32, 8, 512).transpose(2, 0, 1, 3).reshape(128, 16384)
        ).astype(BF)
        m = dict(shared)
        m["p1"] = p1
        in_maps.append(m)
    return in_maps


def _run(inputs, taps=(), **kw):
    _install_bir_fix()
    from concourse.bass_utils import run_bass_kernel_spmd
    key = tuple(sorted(taps))
    if key not in _cache:
        _cache[key] = _build(taps)
    in_maps = _host_prep(inputs)
    return run_bass_kernel_spmd(_cache[key], in_maps, list(range(NCORES)), **kw)


def kernel(**inputs):
    res = _run(inputs)
    out = np.zeros((256, 1, 4, 4), np.float32)
    for c in range(NCORES):
        o = np.asarray(res.results[c]["out"], np.float32).reshape(NPC, 4, 4)
        out[c * NPC:(c + 1) * NPC, 0] = o
    return out
